# revision 1
# baseline (speedup 1.0000x reference)
"""db2 DWT LL band (separable, symmetric pad, stride 2) on Trainium2.

Input  x: (4, 64, 512, 512) f32  ->  Output: (4, 64, 257, 257) f32.

Approach: the 1D low-pass+downsample along an axis is y = x @ M with a banded
matrix M [512, 257] (4 nonzeros per interior column; symmetric-pad edge columns
use combined coefficients). The separable 2D LL band is out = M^T @ img @ M.

On the PE (out = lhsT.T @ rhs, lhsT transposed for free):
  stage A: zT = matmul(lhsT=img, rhs=M)   -> zT[w, h'] (no transpose needed)
  stage B: out = matmul(lhsT=zT,  rhs=M)  -> out[h', w'] (final layout!)

Band sparsity: a 128-row tile of M only covers ~63 output columns, so each
(row-tile, col-range) pair is one matmul with N~64 instead of 257 - a 4x cut
in PE column-cycles, which makes *exact fp32* matmuls (4 cyc/row) affordable.
Row tiles overlap by 2 rows (stride 126) so every output column's 4-tap window
lives in exactly one tile; output column ranges are disjoint, PSUM handles the
scatter via start-once/overwrite semantics.

The leftover output row h'=256 (h' has 257 rows, PE M-dim chunks are 128+128+1)
is computed for all images at once on the DVE from the gathered last two input
rows (symmetric boundary => z = b*x[510] + a*x[511], then the W-filter).

Sharding: pure data parallel - 256 (b,c) images, 32 per core on 8 cores.
"""

import numpy as np
from contextlib import ExitStack

import concourse.bass as bass
import concourse.bacc as bacc
import concourse.tile as tile
from concourse import mybir
from concourse.bass_utils import run_bass_kernel_spmd

F32 = mybir.dt.float32
F32R = mybir.dt.float32r

# db2 dec_lo
H0 = -0.12940952255092145
H1 = 0.22414386804185735
H2 = 0.8365163037378079
H3 = 0.48296291314469025
CA = H1 + H2  # symmetric-edge combined coefficients
CB = H0 + H3

S = 512     # input height/width
O = 257     # output height/width per axis
N_CORES = 8
N_IMG = 32  # images per core (256 total / 8 cores)

# overlapping row tiles (stride 126) and the disjoint output-column range each covers
ROW_TILES = [(0, 128), (126, 254), (252, 380), (378, 506), (504, 512)]
COL_RANGES = [(0, 64), (64, 127), (127, 190), (190, 253), (253, 257)]

MULT = mybir.AluOpType.mult
ADD = mybir.AluOpType.add


def build_filter_matrix() -> np.ndarray:
    m = np.zeros((S, O), dtype=np.float32)
    m[0, 0], m[1, 0] = CA, CB
    for i in range(1, 256):
        m[2 * i - 2, i] = H3
        m[2 * i - 1, i] = H2
        m[2 * i, i] = H1
        m[2 * i + 1, i] = H0
    m[510, 256], m[511, 256] = CB, CA
    return m


def _emit_row256(nc, tc, ctx, x, y, n_img):
    """out[:, 256, :] for all images: z = CB*x[510] + CA*x[511], then W-filter."""
    rp = ctx.enter_context(tc.tile_pool(name="r256", bufs=1))
    r510 = rp.tile([n_img, S], F32, tag="r510")
    nc.sync.dma_start(r510[:], x[:, 510, :].bitcast(F32))
    r511 = rp.tile([n_img, S], F32, tag="r511")
    nc.sync.dma_start(r511[:], x[:, 511, :].bitcast(F32))
    t0 = rp.tile([n_img, S], F32, tag="t0")
    nc.vector.tensor_scalar_mul(t0[:], r510[:], CB)
    z = rp.tile([n_img, S], F32, tag="z")
    nc.vector.scalar_tensor_tensor(z[:], r511[:], CA, t0[:], op0=MULT, op1=ADD)

    a0 = rp.tile([n_img, 255], F32, tag="a0")
    a1 = rp.tile([n_img, 255], F32, tag="a1")
    y256 = rp.tile([n_img, O], F32, tag="y256")
    # interior columns 1..255: y[i] = H3*z[2i-2] + H2*z[2i-1] + H1*z[2i] + H0*z[2i+1]
    nc.vector.tensor_scalar_mul(a0[:], z[:, 0:510:2], H3)
    nc.vector.scalar_tensor_tensor(a1[:], z[:, 1:511:2], H2, a0[:], op0=MULT, op1=ADD)
    nc.vector.scalar_tensor_tensor(a0[:], z[:, 2:512:2], H1, a1[:], op0=MULT, op1=ADD)
    nc.vector.scalar_tensor_tensor(y256[:, 1:256], z[:, 3:512:2], H0, a0[:], op0=MULT, op1=ADD)
    # edge columns
    c0 = rp.tile([n_img, 1], F32, tag="c0")
    nc.vector.tensor_scalar_mul(c0[:], z[:, 0:1], CA)
    nc.vector.scalar_tensor_tensor(y256[:, 0:1], z[:, 1:2], CB, c0[:], op0=MULT, op1=ADD)
    c1 = rp.tile([n_img, 1], F32, tag="c1")
    nc.vector.tensor_scalar_mul(c1[:], z[:, 510:511], CB)
    nc.vector.scalar_tensor_tensor(y256[:, 256:257], z[:, 511:512], CA, c1[:], op0=MULT, op1=ADD)
    nc.sync.dma_start(y[:, 256, :], y256[:])


def build_nc_fp32_banded(n_img: int = N_IMG):
    nc = bacc.Bacc("TRN2", target_bir_lowering=False, debug=False, num_devices=N_CORES)
    x = nc.declare_dram_parameter("x", [n_img, S, S], F32, isOutput=False)
    m = nc.declare_dram_parameter("m", [S, O], F32, isOutput=False)
    y = nc.declare_dram_parameter("y", [n_img, O, O], F32, isOutput=True)

    with tile.TileContext(nc) as tc:
        with ExitStack() as ctx:
            mp = ctx.enter_context(tc.tile_pool(name="mp", bufs=1))
            xp = ctx.enter_context(tc.tile_pool(name="xp", bufs=3))
            zp = ctx.enter_context(tc.tile_pool(name="zp", bufs=2))
            op = ctx.enter_context(tc.tile_pool(name="op", bufs=2))
            psa = ctx.enter_context(tc.tile_pool(name="psa", bufs=1, space="PSUM"))
            psb = ctx.enter_context(tc.tile_pool(name="psb", bufs=1, space="PSUM"))

            # banded filter blocks: mb[j] = M[rows RT[j], cols CR[j]]
            mb = []
            for j, (r0, r1) in enumerate(ROW_TILES):
                c0, c1 = COL_RANGES[j]
                t = mp.tile([r1 - r0, c1 - c0], F32, tag=f"m{j}")
                nc.sync.dma_start(t[:], m[r0:r1, c0:c1])
                mb.append(t)

            prev = None  # (zt tiles, image idx) pending stage B
            for n in range(n_img + 1):
                pz = None
                if n < n_img:
                    xt = []
                    for j, (r0, r1) in enumerate(ROW_TILES):
                        t = xp.tile([r1 - r0, S], F32, tag=f"x{j}")
                        nc.sync.dma_start(t[:], x[n, r0:r1, :])
                        xt.append(t)
                    # stage A: zT tile per w-slice; disjoint col writes per row-tile
                    pz = []
                    for i, (w0, w1) in enumerate(ROW_TILES):
                        p = psa.tile([w1 - w0, 256], F32, tag=f"z{i}")
                        for j, (r0, r1) in enumerate(ROW_TILES):
                            c0, c1 = COL_RANGES[j]
                            c1a = min(c1, 256)
                            nc.tensor.matmul(
                                p[:, c0:c1a], xt[j][:, w0:w1], mb[j][:, 0:c1a - c0],
                                start=(j == 0), stop=(j == len(ROW_TILES) - 1))
                        pz.append(p)
                po = None
                if prev is not None:
                    ztp, _ = prev
                    po = []
                    for mbi in range(2):
                        p = psb.tile([128, O], F32, tag=f"o{mbi}")
                        for i, (w0, w1) in enumerate(ROW_TILES):
                            c0, c1 = COL_RANGES[i]
                            nc.tensor.matmul(
                                p[:, c0:c1], ztp[i][:, mbi * 128:(mbi + 1) * 128], mb[i][:],
                                start=(i == 0), stop=(i == len(ROW_TILES) - 1))
                        po.append(p)
                if n < n_img:
                    zt = []
                    for i, (w0, w1) in enumerate(ROW_TILES):
                        t = zp.tile([w1 - w0, 256], F32, tag=f"zt{i}")
                        eng = nc.vector.tensor_copy if i % 2 == 0 else nc.scalar.copy
                        eng(t[:], pz[i][:])
                        zt.append(t)
                if prev is not None:
                    _, pn = prev
                    for mbi in range(2):
                        t = op.tile([128, O], F32, tag=f"ot{mbi}")
                        eng = nc.scalar.copy if mbi == 0 else nc.vector.tensor_copy
                        eng(t[:], po[mbi][:])
                        nc.sync.dma_start(y[pn, mbi * 128:(mbi + 1) * 128, :], t[:])
                prev = (zt, n) if n < n_img else None

            _emit_row256(nc, tc, ctx, x, y, n_img)
    nc.compile()
    return nc


def build_nc_fp32r_dense(n_img: int = N_IMG):
    nc = bacc.Bacc("TRN2", target_bir_lowering=False, debug=False, num_devices=N_CORES)
    x = nc.declare_dram_parameter("x", [n_img, S, S], F32R, isOutput=False)
    m = nc.declare_dram_parameter("m", [S, O], F32, isOutput=False)
    y = nc.declare_dram_parameter("y", [n_img, O, O], F32, isOutput=True)

    with tile.TileContext(nc) as tc:
        with ExitStack() as ctx:
            mp = ctx.enter_context(tc.tile_pool(name="mp", bufs=1))
            xp = ctx.enter_context(tc.tile_pool(name="xp", bufs=3))
            zp = ctx.enter_context(tc.tile_pool(name="zp", bufs=2))
            op = ctx.enter_context(tc.tile_pool(name="op", bufs=2))
            psa = ctx.enter_context(tc.tile_pool(name="psa", bufs=1, space="PSUM"))
            psb = ctx.enter_context(tc.tile_pool(name="psb", bufs=1, space="PSUM"))

            # M chunks as fp32r (round once via DVE). 258 cols: fp32r moving
            # free-dim must be even; col 257 is zero padding.
            mr = []
            for j in range(4):
                tf = mp.tile([128, O], F32, tag=f"mf{j}")
                nc.sync.dma_start(tf[:], m[j * 128:(j + 1) * 128, :])
                tr = mp.tile([128, 258], F32, tag=f"mr{j}")
                nc.vector.memset(tr[:, 256:258], 0.0)
                nc.vector.tensor_copy(tr[:, 0:257].bitcast(F32R), tf[:])
                mr.append(tr)

            prev = None
            for n in range(n_img + 1):
                pz = None
                if n < n_img:
                    xr = []
                    for j in range(4):
                        t = xp.tile([128, S], F32R, tag=f"x{j}")
                        nc.sync.dma_start(t[:], x[n, j * 128:(j + 1) * 128, :])
                        xr.append(t)
                    pz = []
                    for i in range(4):
                        p = psa.tile([128, 256], F32, tag=f"z{i}")
                        for j in range(4):
                            nc.tensor.matmul(
                                p[:], xr[j][:, i * 128:(i + 1) * 128],
                                mr[j][:, 0:256].bitcast(F32R),
                                start=(j == 0), stop=(j == 3))
                        pz.append(p)
                po = None
                if prev is not None:
                    ztp, _ = prev
                    po = []
                    for mbi in range(2):
                        p = psb.tile([128, 258], F32, tag=f"o{mbi}")
                        for i in range(4):
                            nc.tensor.matmul(
                                p[:], ztp[i][:, mbi * 128:(mbi + 1) * 128].bitcast(F32R),
                                mr[i][:].bitcast(F32R),
                                start=(i == 0), stop=(i == 3))
                        po.append(p)
                if n < n_img:
                    zt = []
                    for i in range(4):
                        t = zp.tile([128, 256], F32, tag=f"zt{i}")
                        eng = nc.vector.tensor_copy if i % 2 == 0 else nc.scalar.copy
                        eng(t[:].bitcast(F32R), pz[i][:])
                        zt.append(t)
                if prev is not None:
                    _, pn = prev
                    for mbi in range(2):
                        t = op.tile([128, O], F32, tag=f"ot{mbi}")
                        eng = nc.scalar.copy if mbi == 0 else nc.vector.tensor_copy
                        eng(t[:], po[mbi][:, 0:257])
                        nc.sync.dma_start(y[pn, mbi * 128:(mbi + 1) * 128, :], t[:])
                prev = (zt, n) if n < n_img else None

            _emit_row256(nc, tc, ctx, x, y, n_img)
    nc.compile()
    return nc


_BUILDERS = {
    "fp32_banded": build_nc_fp32_banded,
    "fp32r_dense": build_nc_fp32r_dense,
}
_NC_CACHE = {}
DEFAULT_MODE = "fp32r_dense"


def round_f32r(a: np.ndarray) -> np.ndarray:
    """Round-to-nearest-even to fp32r's 11 explicit mantissa bits."""
    bits = np.ascontiguousarray(a, dtype=np.float32).view(np.uint32)
    r = bits + np.uint32(0x7FF) + ((bits >> np.uint32(12)) & np.uint32(1))
    r &= np.uint32(0xFFFFF000)
    return r.view(np.float32)


def get_nc(mode: str = "fp32_banded", n_img: int = N_IMG):
    key = (mode, n_img)
    if key not in _NC_CACHE:
        _NC_CACHE[key] = _BUILDERS[mode](n_img)
    return _NC_CACHE[key]


def kernel(x: np.ndarray) -> np.ndarray:
    assert x.shape == (4, 64, S, S), x.shape
    xs = np.ascontiguousarray(x, dtype=np.float32).reshape(256, S, S)
    if DEFAULT_MODE == "fp32r_dense":
        xs = round_f32r(xs)
    mfull = build_filter_matrix()
    nc = get_nc(DEFAULT_MODE, N_IMG)
    in_maps = [
        {"x": xs[c * N_IMG:(c + 1) * N_IMG], "m": mfull} for c in range(N_CORES)
    ]
    res = run_bass_kernel_spmd(nc, in_maps, list(range(N_CORES)))
    out = np.concatenate([res.results[c]["y"] for c in range(N_CORES)], axis=0)
    return out.reshape(4, 64, O, O)

